# revision 1
# baseline (speedup 1.0000x reference)
"""3-layer GCN (PyG-style GCNConv with self-loops + symmetric norm) on 8
Trainium2 NeuronCores.

Distribution (1D graph partitioning):
  - nodes split into 8 contiguous blocks of 6250 rows, one per core
  - edges partitioned by destination core, sorted by destination node
  - 256x256 weights replicated on every core

Per layer, per core:
  1. GEMM: y_c = h_c @ W.T  (PE transpose of h tiles, then 2 accumulating
     matmuls against W.T blocks)
  2. AllGather y_c -> y_table[50000, 256] (+ row 50000 = layer bias)
  3. message passing for the core's ~106k incoming edges:
     - edges sorted by dst, grouped into 128-node dst chunks, packed into
       128-edge tiles; since every node has a self-loop, any 128
       consecutive sorted edges span <= 128 distinct dst rows
     - per chunk, edges are split by src parity into an EVEN and an ODD
       stream; each stream gathers y[src] via dma_gather (int16 indices,
       stride-2-row table views, up to G*128 rows per instruction)
     - selection matrix selT[e, d] = (dst_local[e] == d) * norm[e] built
       on-chip from an iota compare, then PSUM-accumulated matmuls
       out_chunk += selT.T @ msg
     - bias enters as a reserved edge (slot 0 of each chunk's even
       stream) whose selection column is forced to all-ones by a constant
       mask and whose gathered row is the bias vector (table row 50000)
  4. epilogue: relu(+bias already in PSUM), residual add (layers 1,2),
     write h rows back to DRAM
"""

import math
import os

import numpy as np

import concourse.bass as bass
import concourse.mybir as mybir
import concourse.tile as tile
from concourse import bacc
from concourse.bass_utils import run_bass_kernel_spmd
from concourse.masks import make_identity

F32 = mybir.dt.float32
I16 = mybir.dt.int16

N_NODES = 50000
HID = 256
NCORES = 8
NPC = N_NODES // NCORES          # 6250 nodes per core
NCHUNK = math.ceil(NPC / 128)    # 49 dst chunks per core
G = 8                            # edge tiles per gather instruction (dma_gather tops out at 1024 idxs)
PAD_DST = 255.0                  # dst_local sentinel that matches no iota lane
NLAYERS = 3
DEBUG_GEMM_ONLY = False
NSWDGE_QUEUES = 4                # parallel SWDGE descriptor-gen queues
MM_DT = mybir.dt.float32  # float32r would be 4x PE rate but ~2e-4 rel err

_cache = {}


def _pack_stream(flat_idx, flat_dst, flat_nrm, NG):
    """flat_* are [NG*G*128] slot arrays in (tile, slot) order.

    Returns (idxT [NG*128, G*8] int16, dstT [NG*128, G] f32,
    nrmT [NG*128, G] f32) in the per-gather-group on-chip layouts.
    """
    dstT = (
        flat_dst.reshape(NG, G, 128).transpose(0, 2, 1).reshape(NG * 128, G)
    )
    nrmT = (
        flat_nrm.reshape(NG, G, 128).transpose(0, 2, 1).reshape(NG * 128, G)
    )
    idxT = np.zeros((NG * 128, G * 8), dtype=np.int16)
    vals = flat_idx.reshape(NG, G * 128)
    for g in range(NG):
        a16 = vals[g].reshape(G * 8, 16).T  # [16, G*8]; slot i at [i%16, i//16]
        idxT[g * 128 : (g + 1) * 128] = np.tile(a16, (8, 1))
    # pack: per row = [G*8 int16 idx | G f32 dst | G f32 nrm] viewed as int32
    meta = np.zeros((NG * 128, G * 4 + G + G), dtype=np.int32)
    meta[:, : G * 4] = idxT.view(np.int32)
    meta[:, G * 4 : G * 5] = dstT.astype(np.float32).view(np.int32)
    meta[:, G * 5 : G * 6] = nrmT.astype(np.float32).view(np.int32)
    return (meta,)


def _preprocess(edge_index):
    """Edge partitioning by destination + per-core parity-stream layouts."""
    src = np.asarray(edge_index[0], dtype=np.int64)
    dst = np.asarray(edge_index[1], dtype=np.int64)
    loops = np.arange(N_NODES, dtype=np.int64)
    s = np.concatenate([src, loops])
    d = np.concatenate([dst, loops])
    deg = np.bincount(d, minlength=N_NODES).astype(np.float32)
    dinv = (1.0 / np.sqrt(deg)).astype(np.float32)
    norm = (dinv[s] * dinv[d]).astype(np.float32)

    # per (core, chunk, parity) edge lists
    edges = []  # [core][chunk] -> (even(src,dstl,nrm), odd(...))
    cntE = np.zeros((NCORES, NCHUNK), dtype=np.int64)
    cntO = np.zeros((NCORES, NCHUNK), dtype=np.int64)
    for c in range(NCORES):
        lo = c * NPC
        m = (d >= lo) & (d < lo + NPC)
        cs, cd, cn = s[m], (d[m] - lo), norm[m]
        order = np.argsort(cd, kind="stable")
        cs, cd, cn = cs[order], cd[order], cn[order]
        bounds = np.searchsorted(cd, np.arange(0, NCHUNK + 1) * 128)
        rows = []
        for ch in range(NCHUNK):
            a, b = bounds[ch], bounds[ch + 1]
            es, ed, en = cs[a:b], cd[a:b] - ch * 128, cn[a:b]
            ms = es + es // NPC  # row in the allgathered [N+NCORES] table
            pe = (ms % 2) == 0
            ev = (ms[pe] // 2, ed[pe], en[pe])
            od = (ms[~pe] // 2, ed[~pe], en[~pe])
            rows.append((ev, od))
            cntE[c, ch] = pe.sum() + 1  # +1 bias edge
            cntO[c, ch] = (~pe).sum()
        edges.append(rows)

    TE = [int(np.ceil(cntE[:, ch].max() / 128)) for ch in range(NCHUNK)]
    TO = [int(np.ceil(cntO[:, ch].max() / 128)) for ch in range(NCHUNK)]
    tilesE, tilesO = int(np.sum(TE)), int(np.sum(TO))
    NGE, NGO = math.ceil(tilesE / G), math.ceil(tilesO / G)
    startE = np.concatenate([[0], np.cumsum(TE)]).astype(int)
    startO = np.concatenate([[0], np.cumsum(TO)]).astype(int)


    per_core = []
    for c in range(NCORES):
        fiE = np.zeros(NGE * G * 128, dtype=np.int64)  # pad idx: even row 0
        fdE = np.full(NGE * G * 128, PAD_DST, dtype=np.float32)
        fnE = np.zeros(NGE * G * 128, dtype=np.float32)
        fiO = np.zeros(NGO * G * 128, dtype=np.int64)  # pad idx: odd row 0
        fdO = np.full(NGO * G * 128, PAD_DST, dtype=np.float32)
        fnO = np.zeros(NGO * G * 128, dtype=np.float32)
        for ch in range(NCHUNK):
            (eis, eds, ens), (ois, ods, ons) = edges[c][ch]
            p0 = startE[ch] * 128
            L = len(eis) + 1
            fiE[p0 : p0 + L] = np.concatenate([[NPC // 2], eis])
            fdE[p0 + 1 : p0 + L] = eds
            fnE[p0 + 1 : p0 + L] = ens
            p0 = startO[ch] * 128
            L = len(ois)
            fiO[p0 : p0 + L] = ois
            fdO[p0 : p0 + L] = ods
            fnO[p0 : p0 + L] = ons
        per_core.append(
            _pack_stream(fiE, fdE, fnE, NGE) + _pack_stream(fiO, fdO, fnO, NGO)
        )

    sched = (tuple(TE), tuple(TO), tilesE, tilesO, NGE, NGO)
    return sched, per_core


def _build(sched, nlayers=3):
    TE, TO, tilesE, tilesO, NGE, NGO = sched
    nc = bacc.Bacc(
        "TRN2",
        target_bir_lowering=False,
        debug=False,
        num_devices=NCORES,
        num_swdge_queues=NSWDGE_QUEUES,
    )
    x_ap = nc.dram_tensor("x", [NPC, HID], F32, kind="ExternalInput").ap()
    wts = nc.dram_tensor(
        "wts", [2 * nlayers, 128, HID], MM_DT, kind="ExternalInput"
    ).ap()
    bias = nc.dram_tensor("bias", [nlayers, HID], MM_DT, kind="ExternalInput").ap()
    consts = nc.dram_tensor("consts", [128, 256], F32, kind="ExternalInput").ap()
    I32 = mybir.dt.int32
    metE = nc.dram_tensor(
        "metE", [NGE * 128, G * 6], I32, kind="ExternalInput"
    ).ap()
    metO = nc.dram_tensor(
        "metO", [NGO * 128, G * 6], I32, kind="ExternalInput"
    ).ap()
    out_ap = nc.dram_tensor("out", [NPC, HID], F32, kind="ExternalOutput").ap()

    with tile.TileContext(nc) as tc:
        with tc.tile_pool(name="const", bufs=1) as cpool, \
             tc.tile_pool(name="hpool", bufs=1) as hpool, \
             tc.tile_pool(name="work", bufs=3) as work, \
             tc.tile_pool(name="meta", bufs=8) as meta, \
             tc.tile_pool(name="msgp", bufs=8) as msgp, \
             tc.tile_pool(name="eqp", bufs=8) as eqp, \
             tc.tile_pool(name="ptp", bufs=2, space="PSUM") as ptp, \
             tc.tile_pool(name="ypp", bufs=2, space="PSUM") as ypp, \
             tc.tile_pool(name="psp", bufs=4, space="PSUM") as psp, \
             tc.tile_pool(name="dram", bufs=1, space="DRAM") as dram:

            identity = cpool.tile([128, 128], F32)
            make_identity(nc, identity[:])
            cst = cpool.tile([128, 256], F32)
            nc.sync.dma_start(out=cst[:], in_=consts[:])
            iota_sb = cst[:, 0:128]
            mask_sb = cst[:, 128:256]

            wt_sb = cpool.tile([128, 2 * nlayers * HID], MM_DT)
            for i in range(2 * nlayers):
                nc.sync.dma_start(
                    out=wt_sb[:, i * HID : (i + 1) * HID], in_=wts[i]
                )

            # h lives in SBUF, one tile per 128-node chunk, updated in place
            h_sb = [
                hpool.tile([128, HID], F32, tag=f"h{c}", name=f"h_sb{c}")
                for c in range(NCHUNK)
            ]
            for c in range(NCHUNK):
                rows = min(128, NPC - c * 128)
                nc.sync.dma_start(
                    out=h_sb[c][:rows], in_=x_ap[c * 128 : c * 128 + rows, :]
                )

            y_cs = [
                dram.tile([NPC + 1, HID], MM_DT, name=f"y_c{i}")
                for i in range(nlayers)
            ]
            y_tables = [
                dram.tile(
                    [(NPC + 1) * NCORES, HID],
                    MM_DT,
                    addr_space="Shared",
                    name=f"y_table{i}",
                )
                for i in range(nlayers)
            ]
            for l in range(nlayers):
                nc.sync.dma_start(
                    out=y_cs[l][NPC : NPC + 1, :], in_=bias[l : l + 1, :]
                )

            def gemm_chunk(l, c):
                """y_cs[l] rows of chunk c = h_sb[c] @ W_l.T"""
                rows = min(128, NPC - c * 128)
                hT = work.tile([128, HID], MM_DT, tag="hT", name="hT")
                for k in range(2):
                    pt = ptp.tile([128, 128], F32, tag="pt", name="pt")
                    nc.tensor.transpose(
                        out=pt[:, :rows],
                        in_=h_sb[c][:rows, k * 128 : (k + 1) * 128],
                        identity=identity[:rows, :rows],
                    )
                    nc.vector.tensor_copy(
                        out=hT[:, k * 128 : k * 128 + rows], in_=pt[:, :rows]
                    )
                yp = ypp.tile([128, HID], F32, tag="yp", name="yp")
                for k in range(2):
                    nc.tensor.matmul(
                        out=yp[:rows, :],
                        lhsT=hT[:, k * 128 : k * 128 + rows],
                        rhs=wt_sb[:, (2 * l + k) * HID : (2 * l + k + 1) * HID],
                        start=(k == 0),
                        stop=(k == 1),
                    )
                y_sb = work.tile([128, HID], MM_DT, tag="y_sb", name="y_sb")
                nc.vector.tensor_copy(out=y_sb[:rows], in_=yp[:rows, :])
                nc.sync.dma_start(
                    out=y_cs[l][c * 128 : c * 128 + rows, :], in_=y_sb[:rows]
                )

            for c in range(NCHUNK):
                gemm_chunk(0, c)

            for l in range(nlayers):
                y_table = y_tables[l]
                stream_info = {
                    "E": (metE, tilesE, y_table[0::2, :]),
                    "O": (metO, tilesO, y_table[1::2, :]),
                }

                nc.gpsimd.collective_compute(
                    "AllGather",
                    mybir.AluOpType.bypass,
                    replica_groups=[list(range(NCORES))],
                    ins=[y_cs[l][:].opt()],
                    outs=[y_table[:].opt()],
                )

                pos = {"E": 0, "O": 0}
                bufs = {}
                for ci in range(NCHUNK):
                    crows = min(128, NPC - ci * 128)
                    ntot = TE[ci] + TO[ci]
                    ps = psp.tile([128, HID], F32, tag="ps", name="ps")
                    jj = 0
                    for sname, T_s in (("E", TE), ("O", TO)):
                        met_d, tiles_s, view = stream_info[sname]
                        for t in range(T_s[ci]):
                            st = pos[sname]
                            g, col = divmod(st, G)
                            if col == 0:
                                rem = min(G, tiles_s - g * G)
                                met_sb = meta.tile(
                                    [128, G * 6], I32, tag="met_sb", name="met_sb"
                                )
                                nc.sync.dma_start(
                                    out=met_sb[:],
                                    in_=met_d[g * 128 : (g + 1) * 128, :],
                                )
                                idx_sb = met_sb[:, : G * 4].bitcast(I16)
                                dst_sb = met_sb[:, G * 4 : G * 5].bitcast(F32)
                                nrm_sb = met_sb[:, G * 5 : G * 6].bitcast(F32)
                                msg = msgp.tile(
                                    [128, G * HID], MM_DT, tag="msg", name="msg"
                                )
                                nc.gpsimd.dma_gather(
                                    out_ap=msg[:, : rem * HID].rearrange(
                                        "p (g d) -> p g d", g=rem
                                    ),
                                    in_ap=view,
                                    idxs_ap=idx_sb[:, : rem * 8],
                                    num_idxs=rem * 128,
                                    num_idxs_reg=rem * 128,
                                    elem_size=HID,
                                    elem_step=2 * HID,
                                    queue_num=(g + (0 if sname == "E" else 2))
                                    % NSWDGE_QUEUES,
                                )
                                eq = eqp.tile(
                                    [128, G * 128], MM_DT, tag="eq", name="eq"
                                )
                                eq3 = eq[:, : rem * 128].rearrange(
                                    "p (g d) -> p g d", g=rem
                                )
                                nc.vector.tensor_tensor(
                                    out=eq3,
                                    in0=dst_sb[:, :rem, None].to_broadcast(
                                        (128, rem, 128)
                                    ),
                                    in1=iota_sb[:, None, :].to_broadcast(
                                        (128, rem, 128)
                                    ),
                                    op=mybir.AluOpType.is_equal,
                                )
                                nc.vector.tensor_tensor(
                                    out=eq3,
                                    in0=eq3,
                                    in1=nrm_sb[:, :rem, None].to_broadcast(
                                        (128, rem, 128)
                                    ),
                                    op=mybir.AluOpType.mult,
                                )
                                bufs[sname] = (msg, eq)
                            msg, eq = bufs[sname]
                            if sname == "E" and t == 0:
                                # bias edge: force its sel column to ones
                                nc.vector.tensor_tensor(
                                    out=eq[:, col * 128 : (col + 1) * 128],
                                    in0=eq[:, col * 128 : (col + 1) * 128],
                                    in1=mask_sb,
                                    op=mybir.AluOpType.add,
                                )
                            nc.tensor.matmul(
                                out=ps[:, :],
                                lhsT=eq[:, col * 128 : (col + 1) * 128],
                                rhs=msg[:, col * HID : (col + 1) * HID],
                                start=(jj == 0),
                                stop=(jj == ntot - 1),
                            )
                            pos[sname] += 1
                            jj += 1
                    # epilogue: relu (+bias in psum), residual, h update
                    if l == 0:
                        nc.scalar.activation(
                            out=h_sb[ci][:crows],
                            in_=ps[:crows, :],
                            func=mybir.ActivationFunctionType.Relu,
                        )
                    else:
                        o_sb = work.tile([128, HID], F32, tag="o_sb", name="o_sb")
                        nc.scalar.activation(
                            out=o_sb[:crows],
                            in_=ps[:crows, :],
                            func=mybir.ActivationFunctionType.Relu,
                        )
                        if l < nlayers - 1:
                            nc.vector.tensor_add(
                                out=h_sb[ci][:crows],
                                in0=o_sb[:crows],
                                in1=h_sb[ci][:crows],
                            )
                        else:
                            nc.vector.tensor_add(
                                out=o_sb[:crows],
                                in0=o_sb[:crows],
                                in1=h_sb[ci][:crows],
                            )
                            nc.sync.dma_start(
                                out=out_ap[ci * 128 : ci * 128 + crows, :],
                                in_=o_sb[:crows],
                            )
                    if l + 1 < nlayers:
                        gemm_chunk(l + 1, ci)

    nc.compile()
    return nc


def _consts_array():
    consts = np.zeros((128, 256), dtype=np.float32)
    consts[:, 0:128] = np.arange(128, dtype=np.float32)[None, :]
    consts[0, 128:256] = 1.0
    return consts


def kernel(x, edge_index, W0, b0, W1, b1, W2, b2):
    x = np.asarray(x, dtype=np.float32)
    edge_index = np.asarray(edge_index)
    Ws = [np.asarray(w, dtype=np.float32) for w in (W0, W1, W2)]
    bs = [np.asarray(b, dtype=np.float32) for b in (b0, b1, b2)]

    sched, per_core = _preprocess(edge_index)

    key = (sched, NLAYERS, DEBUG_GEMM_ONLY)
    if key not in _cache:
        _cache[key] = _build(sched, nlayers=NLAYERS)
    nc = _cache[key]

    wts = np.stack(
        [w.T[k * 128 : (k + 1) * 128, :] for w in Ws for k in range(2)]
    ).astype(np.float32)
    bias_arr = np.stack(bs).astype(np.float32)
    consts = _consts_array()

    in_maps = []
    for c in range(NCORES):
        mE, mO = per_core[c]
        in_maps.append(
            {
                "x": np.ascontiguousarray(x[c * NPC : (c + 1) * NPC]),
                "wts": wts,
                "bias": bias_arr,
                "consts": consts,
                "metE": mE,
                "metO": mO,
            }
        )

    trace = bool(int(os.environ.get("GCN_TRACE", "0")))
    res = run_bass_kernel_spmd(
        nc, in_maps, core_ids=list(range(NCORES)), trace=trace
    )
    if trace:
        kernel.last_exec_time_ns = res.exec_time_ns
        kernel.last_results = res
    out = np.concatenate([res.results[c]["out"] for c in range(NCORES)], axis=0)
    return out



# revision 7
# speedup vs baseline: 1.1400x; 1.1400x over previous
"""3-layer GCN (PyG-style GCNConv with self-loops + symmetric norm) on 8
Trainium2 NeuronCores.

Distribution (1D graph partitioning):
  - nodes split into 8 contiguous blocks of 6250 rows, one per core
  - edges partitioned by destination core, sorted by destination node
  - 256x256 weights replicated on every core

Per layer, per core:
  1. GEMM: y_c = h_c @ W.T  (PE transpose of h tiles, then 2 accumulating
     fp16 matmuls against W.T blocks); y stored fp16
  2. Segmented AllGather y_c -> y_table[50008, 256] fp16 (+ per-core row
     6250 = layer bias).  The table is split into 4 row segments; each
     segment's AllGather fires as soon as its GEMM chunks finish, so the
     collective overlaps the previous layer's message passing (the last,
     smallest segment is the only serial exposure).
  3. message passing for the core's ~106k incoming edges:
     - edges sorted by dst, grouped into 128-node dst chunks, packed into
       128-edge tiles; since every node has a self-loop, any 128
       consecutive sorted edges span <= 128 distinct dst rows
     - per chunk, edges are split by src parity into an EVEN and an ODD
       stream; each stream gathers y[src] via dma_gather (int16 indices,
       stride-2-row fp16 table views, up to G*128 rows per instruction)
     - selection matrix selT[e, d] = (dst_local[e] == d) * norm[e] built
       on-chip one column at a time with a fused
       tensor_scalar(is_equal, mult) (single DVE pass), then
       PSUM-accumulated fp16 matmuls out_chunk += selT.T @ msg
     - bias enters as a reserved edge (slot 0 of each chunk's even
       stream) whose selection column is forced to all-ones by a constant
       mask and whose gathered row is the bias vector (table row 6250 of
       core 0's block)
  4. epilogue: relu (ACT), residual add (layers 1,2), write h rows back
     to DRAM
"""

import math
import os

import numpy as np

import concourse.bass as bass
import concourse.mybir as mybir
import concourse.tile as tile
from concourse import bacc
from concourse.bass_utils import run_bass_kernel_spmd
from concourse.masks import make_identity

F32 = mybir.dt.float32
F16 = mybir.dt.float16
I16 = mybir.dt.int16
I32 = mybir.dt.int32

N_NODES = 50000
HID = 256
NCORES = 8
NPC = N_NODES // NCORES          # 6250 nodes per core
NCHUNK = math.ceil(NPC / 128)    # 49 dst chunks per core
G = 8                            # edge tiles per gather instruction (dma_gather tops out at 1024 idxs)
PAD_DST = 255.0                  # dst_local sentinel that matches no iota lane
NLAYERS = 3
NSWDGE_QUEUES = 4                # parallel SWDGE descriptor-gen queues
MM_DT = mybir.dt.float16         # table/message/matmul dtype (PSUM accum stays fp32)
# AllGather row segments (chunk-index bounds). Last segment is small so the
# serial tail (last-gemm -> last-AG -> next layer's first gather) is short.
SEG_BOUNDS = (0, 20, 36, 46, NCHUNK)
NSEG = len(SEG_BOUNDS) - 1

_cache = {}


def _pack_stream(flat_idx, flat_dst, flat_nrm, NG):
    """flat_* are [NG*G*128] slot arrays in (tile, slot) order.

    Returns packed meta [NG*128, G*6] int32 rows:
    [G*8 int16 idx | G f32 dst | G f32 nrm].
    """
    dstT = (
        flat_dst.reshape(NG, G, 128).transpose(0, 2, 1).reshape(NG * 128, G)
    )
    nrmT = (
        flat_nrm.reshape(NG, G, 128).transpose(0, 2, 1).reshape(NG * 128, G)
    )
    idxT = np.zeros((NG * 128, G * 8), dtype=np.int16)
    vals = flat_idx.reshape(NG, G * 128)
    for g in range(NG):
        a16 = vals[g].reshape(G * 8, 16).T  # [16, G*8]; slot i at [i%16, i//16]
        idxT[g * 128 : (g + 1) * 128] = np.tile(a16, (8, 1))
    meta = np.zeros((NG * 128, G * 4 + G + G), dtype=np.int32)
    meta[:, : G * 4] = idxT.view(np.int32)
    meta[:, G * 4 : G * 5] = dstT.astype(np.float32).view(np.int32)
    meta[:, G * 5 : G * 6] = nrmT.astype(np.float32).view(np.int32)
    return (meta,)


def _preprocess(edge_index):
    """Edge partitioning by destination + per-core parity-stream layouts."""
    src = np.asarray(edge_index[0], dtype=np.int64)
    dst = np.asarray(edge_index[1], dtype=np.int64)
    loops = np.arange(N_NODES, dtype=np.int64)
    s = np.concatenate([src, loops])
    d = np.concatenate([dst, loops])
    deg = np.bincount(d, minlength=N_NODES).astype(np.float32)
    dinv = (1.0 / np.sqrt(deg)).astype(np.float32)
    norm = (dinv[s] * dinv[d]).astype(np.float32)

    # per (core, chunk, parity) edge lists
    edges = []  # [core][chunk] -> (even(src,dstl,nrm), odd(...))
    cntE = np.zeros((NCORES, NCHUNK), dtype=np.int64)
    cntO = np.zeros((NCORES, NCHUNK), dtype=np.int64)
    for c in range(NCORES):
        lo = c * NPC
        m = (d >= lo) & (d < lo + NPC)
        cs, cd, cn = s[m], (d[m] - lo), norm[m]
        order = np.argsort(cd, kind="stable")
        cs, cd, cn = cs[order], cd[order], cn[order]
        bounds = np.searchsorted(cd, np.arange(0, NCHUNK + 1) * 128)
        rows = []
        for ch in range(NCHUNK):
            a, b = bounds[ch], bounds[ch + 1]
            es, ed, en = cs[a:b], cd[a:b] - ch * 128, cn[a:b]
            ms = es + es // NPC  # row in the allgathered [N+NCORES] table
            pe = (ms % 2) == 0
            ev = (ms[pe] // 2, ed[pe], en[pe])
            od = (ms[~pe] // 2, ed[~pe], en[~pe])
            rows.append((ev, od))
            cntE[c, ch] = pe.sum() + 1  # +1 bias edge
            cntO[c, ch] = (~pe).sum()
        edges.append(rows)

    TE = [int(np.ceil(cntE[:, ch].max() / 128)) for ch in range(NCHUNK)]
    TO = [int(np.ceil(cntO[:, ch].max() / 128)) for ch in range(NCHUNK)]
    tilesE, tilesO = int(np.sum(TE)), int(np.sum(TO))
    NGE, NGO = math.ceil(tilesE / G), math.ceil(tilesO / G)
    startE = np.concatenate([[0], np.cumsum(TE)]).astype(int)
    startO = np.concatenate([[0], np.cumsum(TO)]).astype(int)

    per_core = []
    for c in range(NCORES):
        fiE = np.zeros(NGE * G * 128, dtype=np.int64)  # pad idx: even row 0
        fdE = np.full(NGE * G * 128, PAD_DST, dtype=np.float32)
        fnE = np.zeros(NGE * G * 128, dtype=np.float32)
        fiO = np.zeros(NGO * G * 128, dtype=np.int64)  # pad idx: odd row 0
        fdO = np.full(NGO * G * 128, PAD_DST, dtype=np.float32)
        fnO = np.zeros(NGO * G * 128, dtype=np.float32)
        for ch in range(NCHUNK):
            (eis, eds, ens), (ois, ods, ons) = edges[c][ch]
            p0 = startE[ch] * 128
            L = len(eis) + 1
            fiE[p0 : p0 + L] = np.concatenate([[NPC // 2], eis])
            fdE[p0 + 1 : p0 + L] = eds
            fnE[p0 + 1 : p0 + L] = ens
            p0 = startO[ch] * 128
            L = len(ois)
            fiO[p0 : p0 + L] = ois
            fdO[p0 : p0 + L] = ods
            fnO[p0 : p0 + L] = ons
        per_core.append(
            _pack_stream(fiE, fdE, fnE, NGE) + _pack_stream(fiO, fdO, fnO, NGO)
        )

    sched = (tuple(TE), tuple(TO), tilesE, tilesO, NGE, NGO)
    return sched, per_core


def _build(sched, nlayers=3):
    TE, TO, tilesE, tilesO, NGE, NGO = sched
    nc = bacc.Bacc(
        "TRN2",
        target_bir_lowering=False,
        debug=False,
        num_devices=NCORES,
        num_swdge_queues=NSWDGE_QUEUES,
    )
    x_ap = nc.dram_tensor("x", [NPC, HID], F32, kind="ExternalInput").ap()
    wts = nc.dram_tensor(
        "wts", [2 * nlayers, 128, HID], MM_DT, kind="ExternalInput"
    ).ap()
    bias = nc.dram_tensor("bias", [nlayers, HID], MM_DT, kind="ExternalInput").ap()
    consts = nc.dram_tensor("consts", [128, 256], MM_DT, kind="ExternalInput").ap()
    metE = nc.dram_tensor(
        "metE", [NGE * 128, G * 6], I32, kind="ExternalInput"
    ).ap()
    metO = nc.dram_tensor(
        "metO", [NGO * 128, G * 6], I32, kind="ExternalInput"
    ).ap()
    out_ap = nc.dram_tensor("out", [NPC, HID], F32, kind="ExternalOutput").ap()

    # chunk ranges per AllGather segment; seg s covers y_c rows
    # [SEG_BOUNDS[s]*128, SEG_BOUNDS[s+1]*128) clipped to NPC, last seg +1
    # for the bias row.
    seg_chunks = [range(SEG_BOUNDS[s], SEG_BOUNDS[s + 1]) for s in range(NSEG)]
    seg_rows = []
    for s in range(NSEG):
        r0 = SEG_BOUNDS[s] * 128
        r1 = min(SEG_BOUNDS[s + 1] * 128, NPC)
        if s == NSEG - 1:
            r1 += 1  # bias row
        seg_rows.append((r0, r1))

    with tile.TileContext(nc) as tc:
        with tc.tile_pool(name="const", bufs=1) as cpool, \
             tc.tile_pool(name="hpool", bufs=1) as hpool, \
             tc.tile_pool(name="work", bufs=3) as work, \
             tc.tile_pool(name="meta", bufs=8) as meta, \
             tc.tile_pool(name="msgp", bufs=8) as msgp, \
             tc.tile_pool(name="eqp", bufs=8) as eqp, \
             tc.tile_pool(name="ptp", bufs=2, space="PSUM") as ptp, \
             tc.tile_pool(name="ypp", bufs=2, space="PSUM") as ypp, \
             tc.tile_pool(name="psp", bufs=4, space="PSUM") as psp, \
             tc.tile_pool(name="dram", bufs=1, space="DRAM") as dram:

            identity = cpool.tile([128, 128], F32)
            make_identity(nc, identity[:])
            cst = cpool.tile([128, 256], MM_DT)
            nc.sync.dma_start(out=cst[:], in_=consts[:])
            iota_sb = cst[:, 0:128]
            mask_sb = cst[:, 128:256]

            wt_sb = cpool.tile([128, 2 * nlayers * HID], MM_DT)
            for i in range(2 * nlayers):
                nc.sync.dma_start(
                    out=wt_sb[:, i * HID : (i + 1) * HID], in_=wts[i]
                )

            # h lives in SBUF, one tile per 128-node chunk, updated in place
            h_sb = [
                hpool.tile([128, HID], F32, tag=f"h{c}", name=f"h_sb{c}")
                for c in range(NCHUNK)
            ]
            for c in range(NCHUNK):
                rows = min(128, NPC - c * 128)
                nc.sync.dma_start(
                    out=h_sb[c][:rows], in_=x_ap[c * 128 : c * 128 + rows, :]
                )

            y_cs = [
                dram.tile([NPC + 1, HID], MM_DT, name=f"y_c{i}")
                for i in range(nlayers)
            ]
            y_tables = [
                dram.tile(
                    [(NPC + 1) * NCORES, HID],
                    MM_DT,
                    addr_space="Shared",
                    name=f"y_table{i}",
                )
                for i in range(nlayers)
            ]
            for l in range(nlayers):
                nc.sync.dma_start(
                    out=y_cs[l][NPC : NPC + 1, :], in_=bias[l : l + 1, :]
                )

            def gemm_chunk(l, c):
                """layer-l GEMM for chunk c: y rows = h_sb[c] @ W_l.T (fp16)"""
                rows = min(128, NPC - c * 128)
                hT = work.tile([128, HID], MM_DT, tag="hT", name="hT")
                for k in range(2):
                    pt = ptp.tile([128, 128], F32, tag="pt", name="pt")
                    nc.tensor.transpose(
                        out=pt[:, :rows],
                        in_=h_sb[c][:rows, k * 128 : (k + 1) * 128],
                        identity=identity[:rows, :rows],
                    )
                    nc.scalar.copy(
                        out=hT[:, k * 128 : k * 128 + rows], in_=pt[:, :rows]
                    )
                yp = ypp.tile([128, HID], F32, tag="yp", name="yp")
                for k in range(2):
                    nc.tensor.matmul(
                        out=yp[:rows, :],
                        lhsT=hT[:, k * 128 : k * 128 + rows],
                        rhs=wt_sb[:, (2 * l + k) * HID : (2 * l + k + 1) * HID],
                        start=(k == 0),
                        stop=(k == 1),
                    )
                y_sb = work.tile([128, HID], MM_DT, tag="y_sb", name="y_sb")
                nc.scalar.copy(out=y_sb[:rows], in_=yp[:rows, :])
                nc.sync.dma_start(
                    out=y_cs[l][c * 128 : c * 128 + rows, :], in_=y_sb[:rows]
                )

            def allgather(l):
                nc.gpsimd.collective_compute(
                    "AllGather",
                    mybir.AluOpType.bypass,
                    replica_groups=[list(range(NCORES))],
                    ins=[y_cs[l][:].opt()],
                    outs=[y_tables[l][:].opt()],
                )

            for ci in range(NCHUNK):
                gemm_chunk(0, ci)
            allgather(0)

            for l in range(nlayers):
                y_table = y_tables[l]
                stream_info = {
                    "E": (metE, tilesE, y_table[0::2, :]),
                    "O": (metO, tilesO, y_table[1::2, :]),
                }

                pos = {"E": 0, "O": 0}
                bufs = {}
                if True:
                    for ci in range(NCHUNK):
                        crows = min(128, NPC - ci * 128)
                        ntot = TE[ci] + TO[ci]
                        ps = psp.tile([128, HID], F32, tag="ps", name="ps")
                        jj = 0
                        for sname, T_s in (("E", TE), ("O", TO)):
                            met_d, tiles_s, view = stream_info[sname]
                            for t in range(T_s[ci]):
                                st = pos[sname]
                                g, col = divmod(st, G)
                                if col == 0:
                                    rem = min(G, tiles_s - g * G)
                                    met_sb = meta.tile(
                                        [128, G * 6], I32, tag="met_sb", name="met_sb"
                                    )
                                    nc.sync.dma_start(
                                        out=met_sb[:],
                                        in_=met_d[g * 128 : (g + 1) * 128, :],
                                    )
                                    idx_sb = met_sb[:, : G * 4].bitcast(I16)
                                    dst_sb = met_sb[:, G * 4 : G * 5].bitcast(F32)
                                    nrm_sb = met_sb[:, G * 5 : G * 6].bitcast(F32)
                                    msg = msgp.tile(
                                        [128, G * HID], MM_DT, tag="msg", name="msg"
                                    )
                                    nc.gpsimd.dma_gather(
                                        out_ap=msg[:, : rem * HID].rearrange(
                                            "p (g d) -> p g d", g=rem
                                        ),
                                        in_ap=view,
                                        idxs_ap=idx_sb[:, : rem * 8],
                                        num_idxs=rem * 128,
                                        num_idxs_reg=rem * 128,
                                        elem_size=HID,
                                        elem_step=2 * HID,
                                        queue_num=(g + (0 if sname == "E" else 2))
                                        % NSWDGE_QUEUES,
                                    )
                                    eq = eqp.tile(
                                        [128, G * 128], MM_DT, tag="eq", name="eq"
                                    )
                                    # selT column for each tile: fused
                                    # (iota == dst) * nrm in one DVE pass
                                    for cc in range(rem):
                                        nc.vector.tensor_scalar(
                                            out=eq[:, cc * 128 : (cc + 1) * 128],
                                            in0=iota_sb,
                                            scalar1=dst_sb[:, cc : cc + 1],
                                            scalar2=nrm_sb[:, cc : cc + 1],
                                            op0=mybir.AluOpType.is_equal,
                                            op1=mybir.AluOpType.mult,
                                        )
                                    bufs[sname] = (msg, eq)
                                msg, eq = bufs[sname]
                                if sname == "E" and t == 0:
                                    # bias edge: force its sel column to ones
                                    nc.vector.tensor_tensor(
                                        out=eq[:, col * 128 : (col + 1) * 128],
                                        in0=eq[:, col * 128 : (col + 1) * 128],
                                        in1=mask_sb,
                                        op=mybir.AluOpType.add,
                                    )
                                nc.tensor.matmul(
                                    out=ps[:, :],
                                    lhsT=eq[:, col * 128 : (col + 1) * 128],
                                    rhs=msg[:, col * HID : (col + 1) * HID],
                                    start=(jj == 0),
                                    stop=(jj == ntot - 1),
                                )
                                pos[sname] += 1
                                jj += 1
                        # epilogue: relu (+bias in psum), residual, h update
                        if l == 0:
                            nc.scalar.activation(
                                out=h_sb[ci][:crows],
                                in_=ps[:crows, :],
                                func=mybir.ActivationFunctionType.Relu,
                            )
                        else:
                            o_sb = work.tile([128, HID], F32, tag="o_sb", name="o_sb")
                            nc.scalar.activation(
                                out=o_sb[:crows],
                                in_=ps[:crows, :],
                                func=mybir.ActivationFunctionType.Relu,
                            )
                            if l < nlayers - 1:
                                nc.vector.tensor_add(
                                    out=h_sb[ci][:crows],
                                    in0=o_sb[:crows],
                                    in1=h_sb[ci][:crows],
                                )
                            else:
                                nc.vector.tensor_add(
                                    out=o_sb[:crows],
                                    in0=o_sb[:crows],
                                    in1=h_sb[ci][:crows],
                                )
                                nc.sync.dma_start(
                                    out=out_ap[ci * 128 : ci * 128 + crows, :],
                                    in_=o_sb[:crows],
                                )
                        if l + 1 < nlayers:
                            gemm_chunk(l + 1, ci)
                    if l + 1 < nlayers:
                        allgather(l + 1)

    nc.compile()
    return nc


def _consts_array():
    consts = np.zeros((128, 256), dtype=np.float16)
    consts[:, 0:128] = np.arange(128, dtype=np.float16)[None, :]
    consts[0, 128:256] = 1.0
    return consts


def kernel(x, edge_index, W0, b0, W1, b1, W2, b2):
    x = np.asarray(x, dtype=np.float32)
    edge_index = np.asarray(edge_index)
    Ws = [np.asarray(w, dtype=np.float32) for w in (W0, W1, W2)]
    bs = [np.asarray(b, dtype=np.float32) for b in (b0, b1, b2)]

    sched, per_core = _preprocess(edge_index)

    key = (sched, NLAYERS)
    if key not in _cache:
        _cache[key] = _build(sched, nlayers=NLAYERS)
    nc = _cache[key]

    wts = np.stack(
        [w.T[k * 128 : (k + 1) * 128, :] for w in Ws for k in range(2)]
    ).astype(np.float16)
    bias_arr = np.stack(bs).astype(np.float16)
    consts = _consts_array()

    in_maps = []
    for c in range(NCORES):
        mE, mO = per_core[c]
        in_maps.append(
            {
                "x": np.ascontiguousarray(x[c * NPC : (c + 1) * NPC]),
                "wts": wts,
                "bias": bias_arr,
                "consts": consts,
                "metE": mE,
                "metO": mO,
            }
        )

    trace = bool(int(os.environ.get("GCN_TRACE", "0")))
    res = run_bass_kernel_spmd(
        nc, in_maps, core_ids=list(range(NCORES)), trace=trace
    )
    if trace:
        kernel.last_exec_time_ns = res.exec_time_ns
        kernel.last_results = res
    out = np.concatenate([res.results[c]["out"] for c in range(NCORES)], axis=0)
    return out


# revision 11
# speedup vs baseline: 1.4071x; 1.2343x over previous
"""3-layer GCN (PyG-style GCNConv with self-loops + symmetric norm) on 8
Trainium2 NeuronCores.

Distribution (1D graph partitioning):
  - nodes split into 8 contiguous blocks of 6250 rows, one per core
  - edges partitioned by destination core, sorted by destination node
  - 256x256 weights replicated on every core

The symmetric norm dinv[src]*dinv[dst] is factored out of the per-edge
selection matrix: the src factor is folded into the gathered table rows
(y_table[s] = dinv[s] * y[s], applied for free by the ACT-engine copy
that drains the GEMM PSUM), and the dst factor is applied by the
ACT-engine ReLU epilogue (activation scale operand).  The selection
matrix is then a pure one-hot built in a single DVE is_equal pass.

Per layer, per core:
  1. GEMM: y rows = dinv * (h_c @ W.T) in fp16 (PE transpose of h tiles,
     2 accumulating fp16 matmuls, ACT copy with per-partition dinv scale)
  2. AllGather into a per-core (non-shared) fp16 table [50000, 256],
     split into 4 row segments; each segment's AllGather is issued as
     soon as its GEMM chunks finish, so the collective overlaps the
     previous layer's message passing (only the last, smallest segment
     is serial exposure).
  3. message passing for the core's ~106k incoming edges:
     - edges sorted by dst, grouped into 128-node dst chunks, packed into
       128-edge tiles; every node has a self-loop, so 128 consecutive
       sorted edges span <= 128 distinct dst rows
     - per chunk, edges split by src parity into EVEN/ODD streams; each
       stream gathers table[src] via dma_gather (int16 indices,
       stride-2-row fp16 table views, G*128 rows per instruction)
     - selection matrix selT[e, d] = (dst_local[e] == d) built with one
       broadcast is_equal; PSUM-accumulated fp16 matmuls
       ps += selT.T @ msg
  4. epilogue: relu(dinv_dst * ps) on ACT (plus bias via DVE when bias
     is nonzero), residual add (layers 1,2), h rows back to DRAM
"""

import math
import os

import numpy as np

import concourse.bass as bass
import concourse.mybir as mybir
import concourse.tile as tile
from concourse import bacc
from concourse.bass_utils import run_bass_kernel_spmd
from concourse.masks import make_identity

F32 = mybir.dt.float32
F16 = mybir.dt.float16
I16 = mybir.dt.int16
I32 = mybir.dt.int32

N_NODES = 50000
HID = 256
NCORES = 8
NPC = N_NODES // NCORES          # 6250 nodes per core
NCHUNK = math.ceil(NPC / 128)    # 49 dst chunks per core
G = 8                            # edge tiles per gather instruction (dma_gather tops out at 1024 idxs)
PAD_DST = 255.0                  # dst_local sentinel that matches no iota lane
NLAYERS = 3
NSWDGE_QUEUES = 4                # parallel SWDGE descriptor-gen queues
MM_DT = mybir.dt.float16         # table/message/matmul dtype (PSUM accum stays fp32)
# AllGather row segments (chunk-index bounds). Last segment is small so the
# serial tail (last-gemm -> last-AG -> next layer's first gather) is short.
SEG_BOUNDS = (0, 20, 36, 46, NCHUNK)
NSEG = len(SEG_BOUNDS) - 1

_cache = {}


def _pack_stream(flat_idx, flat_dst, NG):
    """flat_* are [NG*G*128] slot arrays in (tile, slot) order.

    Returns packed meta [NG*128, G*5] int32 rows: [G*8 int16 idx | G f32 dst].
    """
    dstT = (
        flat_dst.reshape(NG, G, 128).transpose(0, 2, 1).reshape(NG * 128, G)
    )
    idxT = np.zeros((NG * 128, G * 8), dtype=np.int16)
    vals = flat_idx.reshape(NG, G * 128)
    for g in range(NG):
        a16 = vals[g].reshape(G * 8, 16).T  # [16, G*8]; slot i at [i%16, i//16]
        idxT[g * 128 : (g + 1) * 128] = np.tile(a16, (8, 1))
    meta = np.zeros((NG * 128, G * 4 + G), dtype=np.int32)
    meta[:, : G * 4] = idxT.view(np.int32)
    meta[:, G * 4 : G * 5] = dstT.astype(np.float32).view(np.int32)
    return (meta,)


def _seg_layout():
    """Segment-major table layout: [seg0 rows of all cores | seg1 ... ].

    Returns (seg_of_chunk, R0, ROWS_S, OFF): per-chunk segment id, each
    segment's first local row, row count per core, and table row offset.
    """
    seg_of_chunk = np.zeros(NCHUNK, dtype=np.int64)
    for ci in range(NCHUNK):
        seg_of_chunk[ci] = next(
            si for si in range(NSEG) if SEG_BOUNDS[si] <= ci < SEG_BOUNDS[si + 1]
        )
    R0 = np.array([SEG_BOUNDS[s] * 128 for s in range(NSEG)], dtype=np.int64)
    R1 = np.array(
        [min(SEG_BOUNDS[s + 1] * 128, NPC) for s in range(NSEG)], dtype=np.int64
    )
    ROWS_S = R1 - R0
    OFF = np.concatenate([[0], np.cumsum(ROWS_S * NCORES)])[:-1]
    return seg_of_chunk, R0, ROWS_S, OFF


def _table_row(nodes):
    """Map global node ids -> rows in the segment-major allgathered table."""
    seg_of_chunk, R0, ROWS_S, OFF = _seg_layout()
    r = nodes % NPC
    c = nodes // NPC
    seg = seg_of_chunk[r // 128]
    return OFF[seg] + c * ROWS_S[seg] + (r - R0[seg])


def _preprocess(edge_index):
    """Edge partitioning by destination + per-core parity-stream layouts."""
    src = np.asarray(edge_index[0], dtype=np.int64)
    dst = np.asarray(edge_index[1], dtype=np.int64)
    loops = np.arange(N_NODES, dtype=np.int64)
    s = np.concatenate([src, loops])
    d = np.concatenate([dst, loops])
    deg = np.bincount(d, minlength=N_NODES).astype(np.float32)
    dinv = (1.0 / np.sqrt(np.maximum(deg, 1e-12))).astype(np.float32)
    dinv[deg == 0] = 0.0

    # per (core, chunk, parity) edge lists; norm factors folded into the
    # table (src side) and the epilogue (dst side)
    trow = _table_row(s)  # src row in the segment-major table
    edges = []  # [core][chunk] -> (even rows/dsts, odd rows/dsts)
    cntE = np.zeros((NCORES, NCHUNK), dtype=np.int64)
    cntO = np.zeros((NCORES, NCHUNK), dtype=np.int64)
    for c in range(NCORES):
        lo = c * NPC
        m = (d >= lo) & (d < lo + NPC)
        cs, cd = trow[m], (d[m] - lo)
        order = np.argsort(cd, kind="stable")
        cs, cd = cs[order], cd[order]
        bounds = np.searchsorted(cd, np.arange(0, NCHUNK + 1) * 128)
        rows = []
        for ch in range(NCHUNK):
            a, b = bounds[ch], bounds[ch + 1]
            es, ed = cs[a:b], cd[a:b] - ch * 128
            pe = (es % 2) == 0
            ev = (es[pe] // 2, ed[pe])
            od = (es[~pe] // 2, ed[~pe])
            rows.append((ev, od))
            cntE[c, ch] = pe.sum()
            cntO[c, ch] = (~pe).sum()
        edges.append(rows)

    TE = [int(np.ceil(max(cntE[:, ch].max(), 1) / 128)) for ch in range(NCHUNK)]
    TO = [int(np.ceil(max(cntO[:, ch].max(), 1) / 128)) for ch in range(NCHUNK)]
    tilesE, tilesO = int(np.sum(TE)), int(np.sum(TO))
    NGE, NGO = math.ceil(tilesE / G), math.ceil(tilesO / G)
    startE = np.concatenate([[0], np.cumsum(TE)]).astype(int)
    startO = np.concatenate([[0], np.cumsum(TO)]).astype(int)

    per_core = []
    for c in range(NCORES):
        fiE = np.zeros(NGE * G * 128, dtype=np.int64)  # pad idx: row 0
        fdE = np.full(NGE * G * 128, PAD_DST, dtype=np.float32)
        fiO = np.zeros(NGO * G * 128, dtype=np.int64)
        fdO = np.full(NGO * G * 128, PAD_DST, dtype=np.float32)
        for ch in range(NCHUNK):
            (eis, eds), (ois, ods) = edges[c][ch]
            p0 = startE[ch] * 128
            fiE[p0 : p0 + len(eis)] = eis
            fdE[p0 : p0 + len(eds)] = eds
            p0 = startO[ch] * 128
            fiO[p0 : p0 + len(ois)] = ois
            fdO[p0 : p0 + len(ods)] = ods
        per_core.append(_pack_stream(fiE, fdE, NGE) + _pack_stream(fiO, fdO, NGO))

    # per-core dinv, laid out [128, NCHUNK] column-per-chunk
    dinv_cols = np.zeros((NCORES, 128, NCHUNK), dtype=np.float32)
    for c in range(NCORES):
        dv = dinv[c * NPC : (c + 1) * NPC]
        pad = np.zeros(NCHUNK * 128, dtype=np.float32)
        pad[: len(dv)] = dv
        dinv_cols[c] = pad.reshape(NCHUNK, 128).T

    sched = (tuple(TE), tuple(TO), tilesE, tilesO, NGE, NGO)
    return sched, per_core, dinv_cols


def _build(sched, nlayers=3, has_bias=False):
    TE, TO, tilesE, tilesO, NGE, NGO = sched
    nc = bacc.Bacc(
        "TRN2",
        target_bir_lowering=False,
        debug=False,
        num_devices=NCORES,
        num_swdge_queues=NSWDGE_QUEUES,
    )
    x_ap = nc.dram_tensor("x", [NPC, HID], F32, kind="ExternalInput").ap()
    wts = nc.dram_tensor(
        "wts", [2 * nlayers, 128, HID], MM_DT, kind="ExternalInput"
    ).ap()
    bias = nc.dram_tensor("bias", [nlayers, HID], F32, kind="ExternalInput").ap()
    consts = nc.dram_tensor("consts", [128, 128], F32, kind="ExternalInput").ap()
    dinv_ap = nc.dram_tensor(
        "dinv", [128, NCHUNK], F32, kind="ExternalInput"
    ).ap()
    metE = nc.dram_tensor(
        "metE", [NGE * 128, G * 5], I32, kind="ExternalInput"
    ).ap()
    metO = nc.dram_tensor(
        "metO", [NGO * 128, G * 5], I32, kind="ExternalInput"
    ).ap()
    out_ap = nc.dram_tensor("out", [NPC, HID], F32, kind="ExternalOutput").ap()

    seg_chunks = [range(SEG_BOUNDS[s], SEG_BOUNDS[s + 1]) for s in range(NSEG)]
    seg_of_chunk_arr, R0_arr, ROWS_S_arr, OFF_arr = _seg_layout()
    seg_rows = [
        (int(R0_arr[s]), int(R0_arr[s] + ROWS_S_arr[s])) for s in range(NSEG)
    ]
    seg_of_chunk = [int(v) for v in seg_of_chunk_arr]
    seg_off = [int(v) for v in OFF_arr]

    with tile.TileContext(nc) as tc:
        with tc.tile_pool(name="const", bufs=1) as cpool, \
             tc.tile_pool(name="hpool", bufs=1) as hpool, \
             tc.tile_pool(name="work", bufs=3) as work, \
             tc.tile_pool(name="meta", bufs=8) as meta, \
             tc.tile_pool(name="msgp", bufs=8) as msgp, \
             tc.tile_pool(name="eqp", bufs=8) as eqp, \
             tc.tile_pool(name="ptp", bufs=2, space="PSUM") as ptp, \
             tc.tile_pool(name="ypp", bufs=2, space="PSUM") as ypp, \
             tc.tile_pool(name="psp", bufs=4, space="PSUM") as psp, \
             tc.tile_pool(name="dram", bufs=1, space="DRAM") as dram:

            identity = cpool.tile([128, 128], F32)
            make_identity(nc, identity[:])
            iota_sb = cpool.tile([128, 128], F32)
            nc.sync.dma_start(out=iota_sb[:], in_=consts[:])
            dinv_sb = cpool.tile([128, NCHUNK], F32)
            nc.sync.dma_start(out=dinv_sb[:], in_=dinv_ap[:])

            wt_sb = cpool.tile([128, 2 * nlayers * HID], MM_DT)
            for i in range(2 * nlayers):
                nc.sync.dma_start(
                    out=wt_sb[:, i * HID : (i + 1) * HID], in_=wts[i]
                )

            if has_bias:
                # bias replicated to all partitions via a k=1 outer product
                bias_row = cpool.tile([1, nlayers * HID], F32)
                ones_sb = cpool.tile([1, 128], F32)
                nc.vector.memset(ones_sb[:], 1.0)
                bias128 = cpool.tile([128, nlayers * HID], F32)
                for l in range(nlayers):
                    nc.sync.dma_start(
                        out=bias_row[:, l * HID : (l + 1) * HID],
                        in_=bias[l : l + 1, :],
                    )
                    bp = ptp.tile([128, HID], F32, tag="pt", name="pt")
                    nc.tensor.matmul(
                        out=bp[:, :],
                        lhsT=ones_sb[:],
                        rhs=bias_row[:, l * HID : (l + 1) * HID],
                        start=True,
                        stop=True,
                    )
                    nc.scalar.copy(
                        out=bias128[:, l * HID : (l + 1) * HID], in_=bp[:, :]
                    )

            # h lives in SBUF, one tile per 128-node chunk, updated in place
            h_sb = [
                hpool.tile([128, HID], F32, tag=f"h{c}", name=f"h_sb{c}")
                for c in range(NCHUNK)
            ]
            for c in range(NCHUNK):
                rows = min(128, NPC - c * 128)
                nc.sync.dma_start(
                    out=h_sb[c][:rows], in_=x_ap[c * 128 : c * 128 + rows, :]
                )

            # per-(layer, segment) AllGather inputs; table is per-core
            # (non-shared) so the 4 segment collectives may each write it
            y_cs = [
                [
                    dram.tile(
                        [seg_rows[s][1] - seg_rows[s][0], HID],
                        MM_DT,
                        name=f"y_c{i}_{s}",
                    )
                    for s in range(NSEG)
                ]
                for i in range(nlayers)
            ]
            y_tables = [
                dram.tile([NPC * NCORES, HID], MM_DT, name=f"y_table{i}")
                for i in range(nlayers)
            ]

            def gemm_chunk(l, c):
                """layer-l GEMM chunk c: y rows = dinv * (h_sb[c] @ W_l.T)"""
                rows = min(128, NPC - c * 128)
                hT = work.tile([128, HID], MM_DT, tag="hT", name="hT")
                for k in range(2):
                    pt = ptp.tile([128, 128], F32, tag="pt", name="pt")
                    nc.tensor.transpose(
                        out=pt[:, :rows],
                        in_=h_sb[c][:rows, k * 128 : (k + 1) * 128],
                        identity=identity[:rows, :rows],
                    )
                    nc.scalar.copy(
                        out=hT[:, k * 128 : k * 128 + rows], in_=pt[:, :rows]
                    )
                yp = ypp.tile([128, HID], F32, tag="yp", name="yp")
                for k in range(2):
                    nc.tensor.matmul(
                        out=yp[:rows, :],
                        lhsT=hT[:, k * 128 : k * 128 + rows],
                        rhs=wt_sb[:, (2 * l + k) * HID : (2 * l + k + 1) * HID],
                        start=(k == 0),
                        stop=(k == 1),
                    )
                y_sb = work.tile([128, HID], MM_DT, tag="y_sb", name="y_sb")
                nc.scalar.activation(
                    out=y_sb[:rows],
                    in_=yp[:rows, :],
                    func=mybir.ActivationFunctionType.Copy,
                    scale=dinv_sb[:rows, c : c + 1],
                )
                s = seg_of_chunk[c]
                base = c * 128 - seg_rows[s][0]
                nc.sync.dma_start(
                    out=y_cs[l][s][base : base + rows, :], in_=y_sb[:rows]
                )

            def allgather_seg(l, s):
                r0, r1 = seg_rows[s]
                nrows = (r1 - r0) * NCORES
                nc.gpsimd.collective_compute(
                    "AllGather",
                    mybir.AluOpType.bypass,
                    replica_groups=[list(range(NCORES))],
                    ins=[y_cs[l][s][:].opt()],
                    outs=[y_tables[l][seg_off[s] : seg_off[s] + nrows, :].opt()],
                )

            for s in range(NSEG):
                for ci in seg_chunks[s]:
                    gemm_chunk(0, ci)
                allgather_seg(0, s)

            for l in range(nlayers):
                y_table = y_tables[l]
                stream_info = {
                    "E": (metE, tilesE, y_table[0::2, :]),
                    "O": (metO, tilesO, y_table[1::2, :]),
                }

                pos = {"E": 0, "O": 0}
                bufs = {}
                for s in range(NSEG):
                    for ci in seg_chunks[s]:
                        crows = min(128, NPC - ci * 128)
                        ntot = TE[ci] + TO[ci]
                        ps = psp.tile([128, HID], F32, tag="ps", name="ps")
                        jj = 0
                        for sname, T_s in (("E", TE), ("O", TO)):
                            met_d, tiles_s, view = stream_info[sname]
                            for t in range(T_s[ci]):
                                st = pos[sname]
                                g, col = divmod(st, G)
                                if col == 0:
                                    rem = min(G, tiles_s - g * G)
                                    met_sb = meta.tile(
                                        [128, G * 5], I32, tag="met_sb", name="met_sb"
                                    )
                                    nc.sync.dma_start(
                                        out=met_sb[:],
                                        in_=met_d[g * 128 : (g + 1) * 128, :],
                                    )
                                    idx_sb = met_sb[:, : G * 4].bitcast(I16)
                                    dst_sb = met_sb[:, G * 4 : G * 5].bitcast(F32)
                                    msg = msgp.tile(
                                        [128, G * HID], MM_DT, tag="msg", name="msg"
                                    )
                                    nc.gpsimd.dma_gather(
                                        out_ap=msg[:, : rem * HID].rearrange(
                                            "p (g d) -> p g d", g=rem
                                        ),
                                        in_ap=view,
                                        idxs_ap=idx_sb[:, : rem * 8],
                                        num_idxs=rem * 128,
                                        num_idxs_reg=rem * 128,
                                        elem_size=HID,
                                        elem_step=2 * HID,
                                        queue_num=(g + (0 if sname == "E" else 2))
                                        % NSWDGE_QUEUES,
                                    )
                                    eq = eqp.tile(
                                        [128, G * 128], MM_DT, tag="eq", name="eq"
                                    )
                                    eq3 = eq[:, : rem * 128].rearrange(
                                        "p (g d) -> p g d", g=rem
                                    )
                                    nc.vector.tensor_tensor(
                                        out=eq3,
                                        in0=dst_sb[:, :rem, None].to_broadcast(
                                            (128, rem, 128)
                                        ),
                                        in1=iota_sb[:, None, :].to_broadcast(
                                            (128, rem, 128)
                                        ),
                                        op=mybir.AluOpType.is_equal,
                                    )
                                    bufs[sname] = (msg, eq)
                                msg, eq = bufs[sname]
                                nc.tensor.matmul(
                                    out=ps[:, :],
                                    lhsT=eq[:, col * 128 : (col + 1) * 128],
                                    rhs=msg[:, col * HID : (col + 1) * HID],
                                    start=(jj == 0),
                                    stop=(jj == ntot - 1),
                                )
                                pos[sname] += 1
                                jj += 1
                        # epilogue: relu(dinv_dst * ps [+ bias]), residual
                        if has_bias:
                            t_sb = work.tile(
                                [128, HID], F32, tag="o_sb", name="t_sb"
                            )
                            nc.vector.scalar_tensor_tensor(
                                out=t_sb[:crows],
                                in0=ps[:crows, :],
                                scalar=dinv_sb[:crows, ci : ci + 1],
                                in1=bias128[:crows, l * HID : (l + 1) * HID],
                                op0=mybir.AluOpType.mult,
                                op1=mybir.AluOpType.add,
                            )
                            relu_in, relu_scale = t_sb, 1.0
                        else:
                            relu_in, relu_scale = ps, dinv_sb[:crows, ci : ci + 1]
                        if l == 0:
                            nc.scalar.activation(
                                out=h_sb[ci][:crows],
                                in_=relu_in[:crows, :],
                                func=mybir.ActivationFunctionType.Relu,
                                scale=relu_scale,
                            )
                        else:
                            o_sb = work.tile([128, HID], F32, tag="o_sb", name="o_sb")
                            nc.scalar.activation(
                                out=o_sb[:crows],
                                in_=relu_in[:crows, :],
                                func=mybir.ActivationFunctionType.Relu,
                                scale=relu_scale,
                            )
                            if l < nlayers - 1:
                                nc.vector.tensor_add(
                                    out=h_sb[ci][:crows],
                                    in0=o_sb[:crows],
                                    in1=h_sb[ci][:crows],
                                )
                            else:
                                nc.vector.tensor_add(
                                    out=o_sb[:crows],
                                    in0=o_sb[:crows],
                                    in1=h_sb[ci][:crows],
                                )
                                nc.sync.dma_start(
                                    out=out_ap[ci * 128 : ci * 128 + crows, :],
                                    in_=o_sb[:crows],
                                )
                        if l + 1 < nlayers:
                            gemm_chunk(l + 1, ci)
                    if l + 1 < nlayers:
                        allgather_seg(l + 1, s)

    nc.compile()
    return nc


def _consts_array():
    consts = np.zeros((128, 128), dtype=np.float32)
    consts[:, :] = np.arange(128, dtype=np.float32)[None, :]
    return consts


def kernel(x, edge_index, W0, b0, W1, b1, W2, b2):
    x = np.asarray(x, dtype=np.float32)
    edge_index = np.asarray(edge_index)
    Ws = [np.asarray(w, dtype=np.float32) for w in (W0, W1, W2)]
    bs = [np.asarray(b, dtype=np.float32) for b in (b0, b1, b2)]
    has_bias = any(np.any(b != 0) for b in bs)

    sched, per_core, dinv_cols = _preprocess(edge_index)

    key = (sched, NLAYERS, has_bias)
    if key not in _cache:
        _cache[key] = _build(sched, nlayers=NLAYERS, has_bias=has_bias)
    nc = _cache[key]

    wts = np.stack(
        [w.T[k * 128 : (k + 1) * 128, :] for w in Ws for k in range(2)]
    ).astype(np.float16)
    bias_arr = np.stack(bs).astype(np.float32)
    consts = _consts_array()

    in_maps = []
    for c in range(NCORES):
        mE, mO = per_core[c]
        in_maps.append(
            {
                "x": np.ascontiguousarray(x[c * NPC : (c + 1) * NPC]),
                "wts": wts,
                "bias": bias_arr,
                "consts": consts,
                "dinv": np.ascontiguousarray(dinv_cols[c]),
                "metE": mE,
                "metO": mO,
            }
        )

    trace = bool(int(os.environ.get("GCN_TRACE", "0")))
    res = run_bass_kernel_spmd(
        nc, in_maps, core_ids=list(range(NCORES)), trace=trace
    )
    if trace:
        kernel.last_exec_time_ns = res.exec_time_ns
        kernel.last_results = res
    out = np.concatenate([res.results[c]["out"] for c in range(NCORES)], axis=0)
    return out


# revision 16
# speedup vs baseline: 1.4530x; 1.0326x over previous
"""3-layer GCN (PyG-style GCNConv with self-loops + symmetric norm) on 8
Trainium2 NeuronCores.

Distribution (1D graph partitioning):
  - nodes split into 8 contiguous blocks of 6250 rows, one per core
  - edges partitioned by destination core, sorted by destination node
  - 256x256 weights replicated on every core

The symmetric norm dinv[src]*dinv[dst] is factored out of the per-edge
selection matrix: the src factor is folded into the gathered table rows
(y_table[s] = dinv[s] * y[s], applied for free by the ACT-engine copy
that drains the GEMM PSUM), and the dst factor is applied by the
ACT-engine ReLU epilogue (activation scale operand).  The selection
matrix is then a pure one-hot built in a single DVE is_equal pass.

Per layer, per core:
  1. GEMM: y rows = dinv * (h_c @ W.T) in fp16 (PE transpose of h tiles,
     2 accumulating fp16 matmuls, ACT copy with per-partition dinv scale)
  2. AllGather into a per-core (non-shared) fp16 table [50000, 256],
     split into 4 row segments; each segment's AllGather is issued as
     soon as its GEMM chunks finish, so the collective overlaps the
     previous layer's message passing (only the last, smallest segment
     is serial exposure).
  3. message passing for the core's ~106k incoming edges:
     - edges sorted by dst, grouped into 128-node dst chunks, packed into
       128-edge tiles; every node has a self-loop, so 128 consecutive
       sorted edges span <= 128 distinct dst rows
     - per chunk, edges split by src parity into EVEN/ODD streams; each
       stream gathers table[src] via dma_gather (int16 indices,
       stride-2-row fp16 table views, G*128 rows per instruction)
     - selection matrix selT[e, d] = (dst_local[e] == d) built with one
       broadcast is_equal; PSUM-accumulated fp16 matmuls
       ps += selT.T @ msg
  4. epilogue: relu(dinv_dst * ps) on ACT (plus bias via DVE when bias
     is nonzero), residual add (layers 1,2), h rows back to DRAM
"""

import math
import os

import numpy as np

import concourse.bass as bass
import concourse.mybir as mybir
import concourse.tile as tile
from concourse import bacc
from concourse.bass_utils import run_bass_kernel_spmd
from concourse.masks import make_identity

F32 = mybir.dt.float32
F16 = mybir.dt.float16
I16 = mybir.dt.int16
I32 = mybir.dt.int32

N_NODES = 50000
HID = 256
NCORES = 8
NPC = N_NODES // NCORES          # 6250 nodes per core
NCHUNK = math.ceil(NPC / 128)    # 49 dst chunks per core
G = 8                            # edge tiles per gather instruction (dma_gather tops out at 1024 idxs)
PAD_DST = 255.0                  # dst_local sentinel that matches no iota lane
NLAYERS = 3
NSWDGE_QUEUES = 4                # parallel SWDGE descriptor-gen queues
MM_DT = mybir.dt.float16         # table/message/matmul dtype (PSUM accum stays fp32)
# AllGather row segments (chunk-index bounds). Last segment is small so the
# serial tail (last-gemm -> last-AG -> next layer's first gather) is short.
SEG_BOUNDS = (0, 20, 36, 46, NCHUNK)
NSEG = len(SEG_BOUNDS) - 1

_cache = {}


def _pack_stream(flat_idx, flat_dst, NG):
    """flat_* are [NG*G*128] slot arrays in (tile, slot) order.

    Returns packed meta [NG*128, G*5] int32 rows: [G*8 int16 idx | G f32 dst].
    Trailing pad slots of each gather group get idx -1 — the Q7 firmware
    trims trailing negative indices before descriptor generation.
    """
    flat_idx = flat_idx.copy()
    pad = flat_dst == PAD_DST
    if bool(int(os.environ.get("GCN_TRIM_PADS", "0"))):
        for g in range(NG):
            a, b = g * G * 128, (g + 1) * G * 128
            k = b
            while k > a and pad[k - 1]:
                k -= 1
            flat_idx[k:b] = -1
    dstT = (
        flat_dst.reshape(NG, G, 128).transpose(0, 2, 1).reshape(NG * 128, G)
    )
    idxT = np.zeros((NG * 128, G * 8), dtype=np.int16)
    vals = flat_idx.reshape(NG, G * 128)
    for g in range(NG):
        a16 = vals[g].reshape(G * 8, 16).T  # [16, G*8]; slot i at [i%16, i//16]
        idxT[g * 128 : (g + 1) * 128] = np.tile(a16, (8, 1))
    meta = np.zeros((NG * 128, G * 4 + G), dtype=np.int32)
    meta[:, : G * 4] = idxT.view(np.int32)
    meta[:, G * 4 : G * 5] = dstT.astype(np.float32).view(np.int32)
    return (meta,)


def _seg_layout():
    """Segment-major table layout: [seg0 rows of all cores | seg1 ... ].

    Returns (seg_of_chunk, R0, ROWS_S, OFF): per-chunk segment id, each
    segment's first local row, row count per core, and table row offset.
    """
    seg_of_chunk = np.zeros(NCHUNK, dtype=np.int64)
    for ci in range(NCHUNK):
        seg_of_chunk[ci] = next(
            si for si in range(NSEG) if SEG_BOUNDS[si] <= ci < SEG_BOUNDS[si + 1]
        )
    R0 = np.array([SEG_BOUNDS[s] * 128 for s in range(NSEG)], dtype=np.int64)
    R1 = np.array(
        [min(SEG_BOUNDS[s + 1] * 128, NPC) for s in range(NSEG)], dtype=np.int64
    )
    ROWS_S = R1 - R0
    OFF = np.concatenate([[0], np.cumsum(ROWS_S * NCORES)])[:-1]
    return seg_of_chunk, R0, ROWS_S, OFF


def _table_row(nodes):
    """Map global node ids -> rows in the segment-major allgathered table."""
    seg_of_chunk, R0, ROWS_S, OFF = _seg_layout()
    r = nodes % NPC
    c = nodes // NPC
    seg = seg_of_chunk[r // 128]
    return OFF[seg] + c * ROWS_S[seg] + (r - R0[seg])


def _preprocess(edge_index):
    """Edge partitioning by destination + per-core parity-stream layouts."""
    src = np.asarray(edge_index[0], dtype=np.int64)
    dst = np.asarray(edge_index[1], dtype=np.int64)
    loops = np.arange(N_NODES, dtype=np.int64)
    s = np.concatenate([src, loops])
    d = np.concatenate([dst, loops])
    deg = np.bincount(d, minlength=N_NODES).astype(np.float32)
    dinv = (1.0 / np.sqrt(np.maximum(deg, 1e-12))).astype(np.float32)
    dinv[deg == 0] = 0.0

    # per (core, chunk, parity) edge lists; norm factors folded into the
    # table (src side) and the epilogue (dst side)
    trow = _table_row(s)  # src row in the segment-major table
    edges = []  # [core][chunk] -> (even rows/dsts, odd rows/dsts)
    cntE = np.zeros((NCORES, NCHUNK), dtype=np.int64)
    cntO = np.zeros((NCORES, NCHUNK), dtype=np.int64)
    for c in range(NCORES):
        lo = c * NPC
        m = (d >= lo) & (d < lo + NPC)
        cs, cd = trow[m], (d[m] - lo)
        order = np.argsort(cd, kind="stable")
        cs, cd = cs[order], cd[order]
        bounds = np.searchsorted(cd, np.arange(0, NCHUNK + 1) * 128)
        rows = []
        for ch in range(NCHUNK):
            a, b = bounds[ch], bounds[ch + 1]
            es, ed = cs[a:b], cd[a:b] - ch * 128
            pe = (es % 2) == 0
            ev = (es[pe] // 2, ed[pe])
            od = (es[~pe] // 2, ed[~pe])
            rows.append((ev, od))
            cntE[c, ch] = pe.sum()
            cntO[c, ch] = (~pe).sum()
        edges.append(rows)

    TE = [int(np.ceil(max(cntE[:, ch].max(), 1) / 128)) for ch in range(NCHUNK)]
    TO = [int(np.ceil(max(cntO[:, ch].max(), 1) / 128)) for ch in range(NCHUNK)]
    tilesE, tilesO = int(np.sum(TE)), int(np.sum(TO))
    NGE, NGO = math.ceil(tilesE / G), math.ceil(tilesO / G)
    startE = np.concatenate([[0], np.cumsum(TE)]).astype(int)
    startO = np.concatenate([[0], np.cumsum(TO)]).astype(int)

    per_core = []
    for c in range(NCORES):
        fiE = np.zeros(NGE * G * 128, dtype=np.int64)  # pad idx: row 0
        fdE = np.full(NGE * G * 128, PAD_DST, dtype=np.float32)
        fiO = np.zeros(NGO * G * 128, dtype=np.int64)
        fdO = np.full(NGO * G * 128, PAD_DST, dtype=np.float32)
        for ch in range(NCHUNK):
            (eis, eds), (ois, ods) = edges[c][ch]
            p0 = startE[ch] * 128
            fiE[p0 : p0 + len(eis)] = eis
            fdE[p0 : p0 + len(eds)] = eds
            p0 = startO[ch] * 128
            fiO[p0 : p0 + len(ois)] = ois
            fdO[p0 : p0 + len(ods)] = ods
        per_core.append(_pack_stream(fiE, fdE, NGE) + _pack_stream(fiO, fdO, NGO))

    # per-core dinv, laid out [128, NCHUNK] column-per-chunk
    dinv_cols = np.zeros((NCORES, 128, NCHUNK), dtype=np.float32)
    for c in range(NCORES):
        dv = dinv[c * NPC : (c + 1) * NPC]
        pad = np.zeros(NCHUNK * 128, dtype=np.float32)
        pad[: len(dv)] = dv
        dinv_cols[c] = pad.reshape(NCHUNK, 128).T

    sched = (tuple(TE), tuple(TO), tilesE, tilesO, NGE, NGO)
    return sched, per_core, dinv_cols


def _build(sched, nlayers=3, has_bias=False):
    TE, TO, tilesE, tilesO, NGE, NGO = sched
    nc = bacc.Bacc(
        "TRN2",
        target_bir_lowering=False,
        debug=False,
        num_devices=NCORES,
        num_swdge_queues=NSWDGE_QUEUES,
    )
    x_ap = nc.dram_tensor("x", [NPC, HID], F32, kind="ExternalInput").ap()
    wts = nc.dram_tensor(
        "wts", [2 * nlayers, 128, HID], MM_DT, kind="ExternalInput"
    ).ap()
    bias = nc.dram_tensor("bias", [nlayers, HID], F32, kind="ExternalInput").ap()
    consts = nc.dram_tensor("consts", [128, 128], F32, kind="ExternalInput").ap()
    dinv_ap = nc.dram_tensor(
        "dinv", [128, NCHUNK], F32, kind="ExternalInput"
    ).ap()
    metE = nc.dram_tensor(
        "metE", [NGE * 128, G * 5], I32, kind="ExternalInput"
    ).ap()
    metO = nc.dram_tensor(
        "metO", [NGO * 128, G * 5], I32, kind="ExternalInput"
    ).ap()
    out_ap = nc.dram_tensor("out", [NPC, HID], F32, kind="ExternalOutput").ap()

    seg_chunks = [range(SEG_BOUNDS[s], SEG_BOUNDS[s + 1]) for s in range(NSEG)]
    seg_of_chunk_arr, R0_arr, ROWS_S_arr, OFF_arr = _seg_layout()
    seg_rows = [
        (int(R0_arr[s]), int(R0_arr[s] + ROWS_S_arr[s])) for s in range(NSEG)
    ]
    seg_of_chunk = [int(v) for v in seg_of_chunk_arr]
    seg_off = [int(v) for v in OFF_arr]

    with tile.TileContext(nc) as tc:
        with tc.tile_pool(name="const", bufs=1) as cpool, \
             tc.tile_pool(name="hpool", bufs=1) as hpool, \
             tc.tile_pool(name="work", bufs=4) as work, \
             tc.tile_pool(name="meta", bufs=10) as meta, \
             tc.tile_pool(name="msgp", bufs=10) as msgp, \
             tc.tile_pool(name="eqp", bufs=10) as eqp, \
             tc.tile_pool(name="ptp", bufs=1, space="PSUM") as ptp, \
             tc.tile_pool(name="ypp", bufs=1, space="PSUM") as ypp, \
             tc.tile_pool(name="psp", bufs=6, space="PSUM") as psp, \
             tc.tile_pool(name="dram", bufs=1, space="DRAM") as dram:

            identity = cpool.tile([128, 128], F32)
            make_identity(nc, identity[:])
            iota_sb = cpool.tile([128, 128], F32)
            nc.sync.dma_start(out=iota_sb[:], in_=consts[:])
            dinv_sb = cpool.tile([128, NCHUNK], F32)
            nc.sync.dma_start(out=dinv_sb[:], in_=dinv_ap[:])

            wt_sb = cpool.tile([128, 2 * nlayers * HID], MM_DT)
            for i in range(2 * nlayers):
                nc.sync.dma_start(
                    out=wt_sb[:, i * HID : (i + 1) * HID], in_=wts[i]
                )

            if has_bias:
                # bias replicated to all partitions via a k=1 outer product
                bias_row = cpool.tile([1, nlayers * HID], F32)
                ones_sb = cpool.tile([1, 128], F32)
                nc.vector.memset(ones_sb[:], 1.0)
                bias128 = cpool.tile([128, nlayers * HID], F32)
                for l in range(nlayers):
                    nc.sync.dma_start(
                        out=bias_row[:, l * HID : (l + 1) * HID],
                        in_=bias[l : l + 1, :],
                    )
                    bp = ptp.tile([128, HID], F32, tag="pt", name="pt")
                    nc.tensor.matmul(
                        out=bp[:, :],
                        lhsT=ones_sb[:],
                        rhs=bias_row[:, l * HID : (l + 1) * HID],
                        start=True,
                        stop=True,
                    )
                    nc.scalar.copy(
                        out=bias128[:, l * HID : (l + 1) * HID], in_=bp[:, :]
                    )

            # zero-init the msg ring so trailing-pad slots (trimmed gathers)
            # never feed NaN bit patterns into the 0-weighted matmul columns
            for _ in range(10):
                mz = msgp.tile([128, G * HID], MM_DT, tag="msg", name="msg")
                nc.vector.memset(mz[:], 0)

            # h lives in SBUF, one tile per 128-node chunk, updated in place
            h_sb = [
                hpool.tile([128, HID], F32, tag=f"h{c}", name=f"h_sb{c}")
                for c in range(NCHUNK)
            ]
            for c in range(NCHUNK):
                rows = min(128, NPC - c * 128)
                nc.sync.dma_start(
                    out=h_sb[c][:rows], in_=x_ap[c * 128 : c * 128 + rows, :]
                )

            # per-(layer, segment) AllGather inputs; table is per-core
            # (non-shared) so the 4 segment collectives may each write it
            y_cs = [
                [
                    dram.tile(
                        [seg_rows[s][1] - seg_rows[s][0], HID],
                        MM_DT,
                        name=f"y_c{i}_{s}",
                    )
                    for s in range(NSEG)
                ]
                for i in range(nlayers)
            ]
            y_tables = [
                dram.tile([NPC * NCORES, HID], MM_DT, name=f"y_table{i}")
                for i in range(nlayers)
            ]

            def gemm_chunk(l, c):
                """layer-l GEMM chunk c: y rows = dinv * (h_sb[c] @ W_l.T)"""
                rows = min(128, NPC - c * 128)
                hT = work.tile([128, HID], MM_DT, tag="hT", name="hT")
                for k in range(2):
                    pt = ptp.tile([128, 128], F32, tag="pt", name="pt")
                    nc.tensor.transpose(
                        out=pt[:, :rows],
                        in_=h_sb[c][:rows, k * 128 : (k + 1) * 128],
                        identity=identity[:rows, :rows],
                    )
                    nc.scalar.copy(
                        out=hT[:, k * 128 : k * 128 + rows], in_=pt[:, :rows]
                    )
                yp = ypp.tile([128, HID], F32, tag="yp", name="yp")
                for k in range(2):
                    nc.tensor.matmul(
                        out=yp[:rows, :],
                        lhsT=hT[:, k * 128 : k * 128 + rows],
                        rhs=wt_sb[:, (2 * l + k) * HID : (2 * l + k + 1) * HID],
                        start=(k == 0),
                        stop=(k == 1),
                    )
                y_sb = work.tile([128, HID], MM_DT, tag="y_sb", name="y_sb")
                nc.scalar.activation(
                    out=y_sb[:rows],
                    in_=yp[:rows, :],
                    func=mybir.ActivationFunctionType.Copy,
                    scale=dinv_sb[:rows, c : c + 1],
                )
                s = seg_of_chunk[c]
                base = c * 128 - seg_rows[s][0]
                nc.sync.dma_start(
                    out=y_cs[l][s][base : base + rows, :], in_=y_sb[:rows]
                )

            def allgather_seg(l, s):
                r0, r1 = seg_rows[s]
                nrows = (r1 - r0) * NCORES
                nc.gpsimd.collective_compute(
                    "AllGather",
                    mybir.AluOpType.bypass,
                    replica_groups=[list(range(NCORES))],
                    ins=[y_cs[l][s][:].opt()],
                    outs=[y_tables[l][seg_off[s] : seg_off[s] + nrows, :].opt()],
                )

            for s in range(NSEG):
                for ci in seg_chunks[s]:
                    gemm_chunk(0, ci)
                allgather_seg(0, s)

            for l in range(nlayers):
                y_table = y_tables[l]
                stream_info = {
                    "E": (metE, tilesE, y_table[0::2, :]),
                    "O": (metO, tilesO, y_table[1::2, :]),
                }

                pos = {"E": 0, "O": 0}
                bufs = {}
                for s in range(NSEG):
                    for ci in seg_chunks[s]:
                        crows = min(128, NPC - ci * 128)
                        ntot = TE[ci] + TO[ci]
                        ps = psp.tile([128, HID], F32, tag="ps", name="ps")
                        jj = 0
                        for sname, T_s in (("E", TE), ("O", TO)):
                            met_d, tiles_s, view = stream_info[sname]
                            for t in range(T_s[ci]):
                                st = pos[sname]
                                g, col = divmod(st, G)
                                if col == 0:
                                    rem = min(G, tiles_s - g * G)
                                    met_sb = meta.tile(
                                        [128, G * 5], I32, tag="met_sb", name="met_sb"
                                    )
                                    nc.sync.dma_start(
                                        out=met_sb[:],
                                        in_=met_d[g * 128 : (g + 1) * 128, :],
                                    )
                                    idx_sb = met_sb[:, : G * 4].bitcast(I16)
                                    dst_sb = met_sb[:, G * 4 : G * 5].bitcast(F32)
                                    msg = msgp.tile(
                                        [128, G * HID], MM_DT, tag="msg", name="msg"
                                    )
                                    nc.gpsimd.dma_gather(
                                        out_ap=msg[:, : rem * HID].rearrange(
                                            "p (g d) -> p g d", g=rem
                                        ),
                                        in_ap=view,
                                        idxs_ap=idx_sb[:, : rem * 8],
                                        num_idxs=rem * 128,
                                        num_idxs_reg=rem * 128,
                                        elem_size=HID,
                                        elem_step=2 * HID,
                                        queue_num=(g + (0 if sname == "E" else 2))
                                        % NSWDGE_QUEUES,
                                    )
                                    eq = eqp.tile(
                                        [128, G * 128], MM_DT, tag="eq", name="eq"
                                    )
                                    eq3 = eq[:, : rem * 128].rearrange(
                                        "p (g d) -> p g d", g=rem
                                    )
                                    nc.vector.tensor_tensor(
                                        out=eq3,
                                        in0=dst_sb[:, :rem, None].to_broadcast(
                                            (128, rem, 128)
                                        ),
                                        in1=iota_sb[:, None, :].to_broadcast(
                                            (128, rem, 128)
                                        ),
                                        op=mybir.AluOpType.is_equal,
                                    )
                                    bufs[sname] = (msg, eq)
                                msg, eq = bufs[sname]
                                nc.tensor.matmul(
                                    out=ps[:, :],
                                    lhsT=eq[:, col * 128 : (col + 1) * 128],
                                    rhs=msg[:, col * HID : (col + 1) * HID],
                                    start=(jj == 0),
                                    stop=(jj == ntot - 1),
                                )
                                pos[sname] += 1
                                jj += 1
                        # epilogue: relu(dinv_dst * ps [+ bias]), residual
                        if has_bias:
                            t_sb = work.tile(
                                [128, HID], F32, tag="o_sb", name="t_sb"
                            )
                            nc.vector.scalar_tensor_tensor(
                                out=t_sb[:crows],
                                in0=ps[:crows, :],
                                scalar=dinv_sb[:crows, ci : ci + 1],
                                in1=bias128[:crows, l * HID : (l + 1) * HID],
                                op0=mybir.AluOpType.mult,
                                op1=mybir.AluOpType.add,
                            )
                            relu_in, relu_scale = t_sb, 1.0
                        else:
                            relu_in, relu_scale = ps, dinv_sb[:crows, ci : ci + 1]
                        if l == 0:
                            nc.scalar.activation(
                                out=h_sb[ci][:crows],
                                in_=relu_in[:crows, :],
                                func=mybir.ActivationFunctionType.Relu,
                                scale=relu_scale,
                            )
                        else:
                            o_sb = work.tile([128, HID], F32, tag="o_sb", name="o_sb")
                            nc.scalar.activation(
                                out=o_sb[:crows],
                                in_=relu_in[:crows, :],
                                func=mybir.ActivationFunctionType.Relu,
                                scale=relu_scale,
                            )
                            if l < nlayers - 1:
                                nc.vector.tensor_add(
                                    out=h_sb[ci][:crows],
                                    in0=o_sb[:crows],
                                    in1=h_sb[ci][:crows],
                                )
                            else:
                                nc.vector.tensor_add(
                                    out=o_sb[:crows],
                                    in0=o_sb[:crows],
                                    in1=h_sb[ci][:crows],
                                )
                                nc.sync.dma_start(
                                    out=out_ap[ci * 128 : ci * 128 + crows, :],
                                    in_=o_sb[:crows],
                                )
                        if l + 1 < nlayers:
                            gemm_chunk(l + 1, ci)
                    if l + 1 < nlayers:
                        allgather_seg(l + 1, s)

    nc.compile()
    return nc


def _consts_array():
    consts = np.zeros((128, 128), dtype=np.float32)
    consts[:, :] = np.arange(128, dtype=np.float32)[None, :]
    return consts


def kernel(x, edge_index, W0, b0, W1, b1, W2, b2):
    x = np.asarray(x, dtype=np.float32)
    edge_index = np.asarray(edge_index)
    Ws = [np.asarray(w, dtype=np.float32) for w in (W0, W1, W2)]
    bs = [np.asarray(b, dtype=np.float32) for b in (b0, b1, b2)]
    has_bias = any(np.any(b != 0) for b in bs)

    sched, per_core, dinv_cols = _preprocess(edge_index)

    key = (sched, NLAYERS, has_bias)
    if key not in _cache:
        _cache[key] = _build(sched, nlayers=NLAYERS, has_bias=has_bias)
    nc = _cache[key]

    wts = np.stack(
        [w.T[k * 128 : (k + 1) * 128, :] for w in Ws for k in range(2)]
    ).astype(np.float16)
    bias_arr = np.stack(bs).astype(np.float32)
    consts = _consts_array()

    in_maps = []
    for c in range(NCORES):
        mE, mO = per_core[c]
        in_maps.append(
            {
                "x": np.ascontiguousarray(x[c * NPC : (c + 1) * NPC]),
                "wts": wts,
                "bias": bias_arr,
                "consts": consts,
                "dinv": np.ascontiguousarray(dinv_cols[c]),
                "metE": mE,
                "metO": mO,
            }
        )

    trace = bool(int(os.environ.get("GCN_TRACE", "0")))
    res = run_bass_kernel_spmd(
        nc, in_maps, core_ids=list(range(NCORES)), trace=trace
    )
    if trace:
        kernel.last_exec_time_ns = res.exec_time_ns
        kernel.last_results = res
    out = np.concatenate([res.results[c]["out"] for c in range(NCORES)], axis=0)
    return out
